# revision 31
# baseline (speedup 1.0000x reference)
"""AttentionSequencePoolingLayer Trainium2 kernel (8-core data parallel), v2.

B=2048, S=200, D=64, H1=64, H2=16. Batch sharded 256/core.

Strategy:
- Rows are globally sorted by seq_length and dealt round-robin to the 8 cores,
  so every core sees the same length profile. Within a core, rows are grouped
  16 at a time; group g only processes T_g = roundup(max seq_length, 16)
  tokens (ΣT ≈ 0.57 × S). One program (compiled per schedule) serves all cores.
- Per 16-row group: cast-load k to bf16 (tokens on partitions), xbar-transpose
  to kT [(bhat,d), tok] pair tiles.
- z1 = x1 = att@W1 lands in PSUM via 3 accumulating matmuls per pair:
  qW ⊗ ones (K=1 rank-1), Wk^T kT, (q∘Wqk)^T kT. The qW fold makes the
  dice-1 sigmoid bias a shared constant, so ACT/DVE ops batch across pairs.
- dice1, two routes (mixed 1:4 r2b:r1 to balance ACT vs DVE):
  r1: ACT p1=σ(s·x1+b); DVE p1~=α+(1-α)p1; DVE h1=x1(PSUM)∘p1~; z2=W2^T h1.
  r2b: ACT p1; ACT copy x1→SBUF; DVE u1=x1∘p1; z2 = (αW2)^T x1 + ((1-α)W2)^T u1.
- dice2 batched over all 8 pairs of a group [128=(pair,bhat,h2), 2T].
- scores via tiny N=2 matmuls into token-major PSUM; σ+mask; pooling as
  out[d,1] = k_nat^T w with N=1 matmuls (k stationary); d-major DRAM output,
  transposed on host at gather.
"""
import numpy as np
import ml_dtypes

import concourse.bacc as bacc
import concourse.tile as tile
import concourse.mybir as mybir
import concourse.bass as bass
from concourse.bass_utils import run_bass_kernel_spmd

B, S, D = 2048, 200, 64
H1, H2 = 64, 16
EPS = 1e-9
NCORES = 8
BLOC = B // NCORES          # 256 rows per core
NGROUPS = BLOC // 16        # 16

F32 = mybir.dt.float32
BF16 = mybir.dt.bfloat16
AF = mybir.ActivationFunctionType
ALU = mybir.AluOpType
bf = ml_dtypes.bfloat16

# fraction of z1-units on route-2b (ACT-heavy) vs route-1 (DVE-heavy): num/den
R2B_NUM, R2B_DEN = 1, 4
LP_BUFS, KT_BUFS, WP_BUFS, H2_BUFS = 4, 8, 4, 3
PS1_BUFS, PS2_BUFS, PS3_BUFS = 3, 2, 3
WQ_ACT_MOD = 9  # j%4 value routed to ACT; 9=never
UP8, UP4 = 56, 112

_CACHE = {}
TRACE = False
LAST_RESULT = None


def _ceil16(x):
    return (int(x) + 15) // 16 * 16


def _build(sched):
    nc = bacc.Bacc("TRN2", target_bir_lowering=False, debug=False, num_devices=NCORES,
                   dynamic_dma_scratch_size=16384)
    nb = BLOC
    npair = nb // 2  # 128

    key = nc.dram_tensor("key", [nb * S, D], F32, kind="ExternalInput").ap()
    qp = nc.dram_tensor("qp", [128, npair], F32, kind="ExternalInput").ap()
    qw1 = nc.dram_tensor("qw1", [1, 128 * npair], BF16, kind="ExternalInput").ap()
    maskd = nc.dram_tensor("maskd", [128, 32 * NGROUPS], BF16, kind="ExternalInput").ap()
    wk2 = nc.dram_tensor("wk2", [128, 128], BF16, kind="ExternalInput").ap()
    wqk2 = nc.dram_tensor("wqk2", [128, 128], BF16, kind="ExternalInput").ap()
    w2b = nc.dram_tensor("w2b", [128, 32], BF16, kind="ExternalInput").ap()
    w2a = nc.dram_tensor("w2a", [128, 32], BF16, kind="ExternalInput").ap()
    w2na = nc.dram_tensor("w2na", [128, 32], BF16, kind="ExternalInput").ap()
    w34 = nc.dram_tensor("w34", [128, 2], BF16, kind="ExternalInput").ap()
    colsb = nc.dram_tensor("colsb", [128, 8], F32, kind="ExternalInput").ap()
    outd = nc.dram_tensor("out", [D, nb], F32, kind="ExternalOutput").ap()

    key_r = key.rearrange("(b s) d -> s b d", s=S)  # [200, nb, 64] view

    with tile.TileContext(nc) as tc:
        with (
            tc.tile_pool(name="const", bufs=1) as cp,
            tc.tile_pool(name="load", bufs=LP_BUFS) as lp,
            tc.tile_pool(name="kt", bufs=KT_BUFS) as ktp,
            tc.tile_pool(name="work", bufs=WP_BUFS) as wp,
            tc.tile_pool(name="h2p", bufs=H2_BUFS) as h2p,
            tc.tile_pool(name="outp", bufs=2) as op_,
            tc.tile_pool(name="ps1", bufs=PS1_BUFS, space="PSUM") as ps1,
            tc.tile_pool(name="ps2", bufs=PS2_BUFS, space="PSUM") as ps2,
            tc.tile_pool(name="ps3", bufs=PS3_BUFS, space="PSUM") as ps3,
        ):
            # ---- constants into SBUF
            c_qp = cp.tile([128, npair], F32)
            nc.sync.dma_start(out=c_qp[:], in_=qp)
            c_qw1 = cp.tile([1, 128 * npair], BF16)
            nc.sync.dma_start(out=c_qw1[:], in_=qw1)
            c_mask = cp.tile([128, 32 * NGROUPS], BF16)
            nc.sync.dma_start(out=c_mask[:], in_=maskd)
            c_wk = cp.tile([128, 128], BF16)
            nc.sync.dma_start(out=c_wk[:], in_=wk2)
            c_wqk = cp.tile([128, 128], BF16)
            nc.sync.dma_start(out=c_wqk[:], in_=wqk2)
            c_w2b = cp.tile([128, 32], BF16)
            nc.sync.dma_start(out=c_w2b[:], in_=w2b)
            c_w2a = cp.tile([128, 32], BF16)
            nc.sync.dma_start(out=c_w2a[:], in_=w2a)
            c_w2na = cp.tile([128, 32], BF16)
            nc.sync.dma_start(out=c_w2na[:], in_=w2na)
            c_w34 = cp.tile([128, 2], BF16)
            nc.sync.dma_start(out=c_w34[:], in_=w34)
            c_cols = cp.tile([128, 8], F32)
            nc.sync.dma_start(out=c_cols[:], in_=colsb)
            c_ones = cp.tile([1, 128], BF16)
            nc.gpsimd.memset(c_ones[:], 1.0)


            unit_ctr = 0
            gf_by_lg = {}
            gp_by_lg = {}
            kt_by_lg = {}
            po_sb = None
            for gi, g in enumerate(range(NGROUPS)):
                T1, T2 = sched[g]
                Teff = T1 + T2
                lb = g // 4              # 64-row load block
                lg = g // 2              # 32-row transpose pair
                ro = 16 * (g % 4)        # row offset within load block
                rbase = 64 * lb

                if lb not in gf_by_lg:
                    TL1 = max(sched[i][0] for i in range(4 * lb, 4 * lb + 4))
                    TL2 = max(sched[i][1] for i in range(4 * lb, 4 * lb + 4))
                    gf = lp.tile([128, 64, 64], BF16, tag="gf")
                    nc.gpsimd.dma_start(
                        out=gf[0:TL1, :, :], in_=key_r[0:TL1, rbase : rbase + 64, :]
                    )
                    if TL2 > 0:
                        gp = lp.tile([80, 64, 64], BF16, tag="gp")
                        load_rows = TL2
                        if lb == NGROUPS // 4 - 1 and TL2 >= 80:
                            # last rows would over-read past the key tensor
                            nc.gpsimd.memset(gp[64:80, 48:64, :], 0.0)
                            load_rows = 72
                        nc.gpsimd.dma_start(
                            out=gp[0:load_rows, :, :],
                            in_=bass.AP(
                                key.tensor,
                                (rbase * S + 128) * D,
                                [[D, load_rows], [S * D, 64], [1, D]],
                            ),
                        )
                        gp_by_lg[lb] = gp
                    gf_by_lg[lb] = gf
                gf = gf_by_lg[lb]
                gp = gp_by_lg.get(lb)

                # ---- transposes at 32-row (2-group) granularity
                if lg not in kt_by_lg:
                    T1lg = max(sched[2 * lg][0], sched[2 * lg + 1][0])
                    T2lg = max(sched[2 * lg][1], sched[2 * lg + 1][1])
                    tro = 32 * (lg % 2)
                    eng1 = nc.sync if lg % 2 == 0 else nc.scalar
                    eng2 = nc.scalar if lg % 2 == 0 else nc.sync
                    ktf = ktp.tile([128, 16, 128], BF16, tag="ktf")
                    eng1.dma_start(
                        out=ktf[:, :, 0:T1lg],
                        in_=gf[0:T1lg, tro : tro + 32, :].rearrange("p b d -> p (b d)"),
                        transpose=True,
                    )
                    ktq = None
                    if T2lg > 0:
                        ktq = ktp.tile([128, 16, 80], BF16, tag="ktq")
                        eng2.dma_start(
                            out=ktq[:, :, 0:T2lg],
                            in_=gp[0:T2lg, tro : tro + 32, :].rearrange("p b d -> p (b d)"),
                            transpose=True,
                        )
                    kt_by_lg[lg] = (ktf, ktq)
                ktf, ktq = kt_by_lg[lg]
                jo = 8 * (g % 2)         # pair index offset within ktf/ktq

                # ---- per-pair q∘Wqk
                wq = wp.tile([128, 8, 128], BF16, tag="wq")
                for j in range(8):
                    pj = 8 * g + j
                    if j % 4 == WQ_ACT_MOD:
                        nc.scalar.activation(wq[:, j, :], c_wqk[:], AF.Copy,
                                             scale=c_qp[:, pj : pj + 1])
                    else:
                        nc.vector.tensor_scalar(
                            wq[:, j, :], c_wqk[:], c_qp[:, pj : pj + 1], None, ALU.mult
                        )

                # ---- layer 1 + dice1 in units of `up` pairs
                up = 8 if Teff <= UP8 else (4 if Teff <= UP4 else 2)
                z2 = ps2.tile([128, 416], F32, tag="z2")
                for u in range(8 // up):
                    C = up * Teff
                    z1 = ps1.tile([128, 448], F32, tag="z1")
                    for jj in range(up):
                        j = u * up + jj
                        pj = 8 * g + j
                        o = jj * Teff
                        qwj = c_qw1[0:1, 128 * pj : 128 * pj + 128]
                        nc.tensor.matmul(z1[:, o : o + T1], qwj, c_ones[0:1, 0:T1],
                                         start=True, stop=False)
                        nc.tensor.matmul(z1[:, o : o + T1], c_wk[:], ktf[:, jo + j, 0:T1],
                                         start=False, stop=False)
                        nc.tensor.matmul(z1[:, o : o + T1], wq[:, j, :], ktf[:, jo + j, 0:T1],
                                         start=False, stop=True)
                        if T2 > 0:
                            nc.tensor.matmul(z1[:, o + T1 : o + Teff], qwj,
                                             c_ones[0:1, 0:T2], start=True, stop=False)
                            nc.tensor.matmul(z1[:, o + T1 : o + Teff], c_wk[:],
                                             ktq[:, jo + j, 0:T2], start=False, stop=False)
                            nc.tensor.matmul(z1[:, o + T1 : o + Teff], wq[:, j, :],
                                             ktq[:, jo + j, 0:T2], start=False, stop=True)

                    p1t = wp.tile([128, 448], BF16, tag="p1")
                    nc.scalar.activation(p1t[:, 0:C], z1[:, 0:C], AF.Sigmoid,
                                         bias=c_cols[:, 1:2], scale=c_cols[:, 0:1])
                    r2b = (unit_ctr * R2B_NUM) % R2B_DEN < R2B_NUM
                    unit_ctr += 1
                    if r2b:
                        x1c = wp.tile([128, 448], BF16, tag="x1c")
                        nc.scalar.copy(x1c[:, 0:C], z1[:, 0:C])
                        u1t = wp.tile([128, 448], BF16, tag="u1")
                        nc.vector.tensor_tensor(u1t[:, 0:C], x1c[:, 0:C], p1t[:, 0:C],
                                                ALU.mult)
                        for jj in range(up):
                            j = u * up + jj
                            o = jj * Teff
                            b = j // 2
                            co = (j % 2) * Teff
                            nc.tensor.matmul(z2[32 * b : 32 * b + 32, co : co + Teff],
                                             c_w2a[:], x1c[:, o : o + Teff],
                                             start=True, stop=False,
                                             tile_position=(0, 32 * b))
                            nc.tensor.matmul(z2[32 * b : 32 * b + 32, co : co + Teff],
                                             c_w2na[:], u1t[:, o : o + Teff],
                                             start=False, stop=True,
                                             tile_position=(0, 32 * b))
                    else:
                        p1m = wp.tile([128, 448], BF16, tag="p1m")
                        nc.vector.tensor_scalar(p1m[:, 0:C], p1t[:, 0:C],
                                                c_cols[:, 2:3], c_cols[:, 3:4],
                                                ALU.mult, ALU.add)
                        h1t = wp.tile([128, 448], BF16, tag="h1")
                        nc.vector.tensor_tensor(h1t[:, 0:C], z1[:, 0:C], p1m[:, 0:C],
                                                ALU.mult)
                        for jj in range(up):
                            j = u * up + jj
                            o = jj * Teff
                            b = j // 2
                            co = (j % 2) * Teff
                            nc.tensor.matmul(z2[32 * b : 32 * b + 32, co : co + Teff],
                                             c_w2b[:], h1t[:, o : o + Teff],
                                             start=True, stop=True,
                                             tile_position=(0, 32 * b))

                # ---- dice2, batched over the whole group
                C2 = 2 * Teff
                p2t = wp.tile([128, 448], BF16, tag="p2")
                nc.scalar.activation(p2t[:, 0:C2], z2[:, 0:C2], AF.Sigmoid,
                                     bias=c_cols[:, 5:6], scale=c_cols[:, 4:5])
                t2t = wp.tile([128, 448], BF16, tag="t2")
                nc.vector.tensor_scalar(t2t[:, 0:C2], p2t[:, 0:C2], c_cols[:, 6:7],
                                        c_cols[:, 7:8], ALU.mult, ALU.add)
                h2t = h2p.tile([128, 448], BF16, tag="h2")
                nc.vector.tensor_tensor(h2t[:, 0:C2], z2[:, 0:C2], t2t[:, 0:C2],
                                        ALU.mult)

                # ---- scores (token-major PSUM [128, 32])
                sc = ps3.tile([128, 48], F32, tag="sc")
                nc.vector.memset(sc[:, 0:32], 0.0)
                for j in range(8):
                    b = j // 2
                    co = (j % 2) * Teff
                    nc.tensor.matmul(sc[0:T1, 4 * j : 4 * j + 2],
                                     h2t[32 * b : 32 * b + 32, co : co + T1],
                                     c_w34[32 * b : 32 * b + 32, :],
                                     start=True, stop=True,
                                     tile_position=(32 * b, 0))
                    if T2 > 0:
                        nc.tensor.matmul(sc[0:T2, 4 * j + 2 : 4 * j + 4],
                                         h2t[32 * b : 32 * b + 32, co + T1 : co + Teff],
                                         c_w34[32 * b : 32 * b + 32, :],
                                         start=True, stop=True,
                                         tile_position=(32 * b, 0))

                sg = wp.tile([128, 32], BF16, tag="sg")
                nc.scalar.activation(sg[:], sc[:, 0:32], AF.Sigmoid)
                wt = wp.tile([128, 32], BF16, tag="wt")
                nc.vector.tensor_tensor(wt[:], sg[:], c_mask[:, 32 * g : 32 * g + 32],
                                        ALU.mult)

                # ---- pooling: out[d, row] = k^T w, N=1 matmuls
                po = sc[0:64, 32:48]
                for r in range(16):
                    j = r // 2
                    bh = r % 2
                    nc.tensor.matmul(po[:, r : r + 1], gf[0:T1, ro + r, :],
                                     wt[0:T1, 4 * j + bh : 4 * j + bh + 1],
                                     start=True, stop=(T2 == 0))
                    if T2 > 0:
                        nc.tensor.matmul(po[:, r : r + 1], gp[0:T2, ro + r, :],
                                         wt[0:T2, 4 * j + 2 + bh : 4 * j + 3 + bh],
                                         start=False, stop=True)

                if gi % 4 == 0:
                    po_sb = op_.tile([64, 64], F32, tag="posb")
                nc.vector.tensor_copy(po_sb[:, 16 * (g % 4) : 16 * (g % 4) + 16], po)
                if gi % 4 == 3:
                    nc.sync.dma_start(
                        out=outd[:, 64 * (g // 4) : 64 * (g // 4) + 64], in_=po_sb[:]
                    )
    nc.compile()
    return nc


def _blk(a):
    m = np.zeros((128, 2 * a.shape[1]), np.float32)
    m[0:64, 0 : a.shape[1]] = a
    m[64:128, a.shape[1] :] = a
    return m


def _prep_consts(W1, alpha1, mean1, var1, W2, alpha2, mean2, var2, W3):
    inv1 = 1.0 / np.sqrt(var1 + EPS)
    inv2 = 1.0 / np.sqrt(var2 + EPS)
    Wq = W1[0:64] + W1[128:192]
    Wk = W1[64:128] - W1[128:192]
    Wqk = W1[192:256]

    wk2 = _blk(Wk).astype(bf)
    wqk2 = _blk(Wqk).astype(bf)
    w2b = _blk(W2).astype(bf)
    w2a = _blk(np.diag(alpha1) @ W2).astype(bf)
    w2na = _blk(np.diag(1.0 - alpha1) @ W2).astype(bf)
    w34p = np.zeros((32, 2), np.float32)
    w34p[0:16, 0] = W3[:, 0]
    w34p[16:32, 1] = W3[:, 0]
    w34 = np.tile(w34p, (4, 1)).astype(bf)
    colsb = np.zeros((128, 8), np.float32)
    colsb[:, 0] = np.tile(inv1, 2)
    colsb[:, 1] = np.tile(-mean1 * inv1, 2)
    colsb[:, 2] = np.tile(1.0 - alpha1, 2)
    colsb[:, 3] = np.tile(alpha1, 2)
    colsb[:, 4] = np.tile(inv2, 8)
    colsb[:, 5] = np.tile(-mean2 * inv2, 8)
    colsb[:, 6] = np.tile(1.0 - alpha2, 8)
    colsb[:, 7] = np.tile(alpha2, 8)
    return Wq, wk2, wqk2, w2b, w2a, w2na, w34, colsb


def kernel(query_emb, key_emb, seq_length, W1, alpha1, mean1, var1,
           W2, alpha2, mean2, var2, W3):
    (Wq, wk2, wqk2, w2b, w2a, w2na, w34, colsb) = _prep_consts(
        np.asarray(W1, np.float32), np.asarray(alpha1, np.float32),
        np.asarray(mean1, np.float32), np.asarray(var1, np.float32),
        np.asarray(W2, np.float32), np.asarray(alpha2, np.float32),
        np.asarray(mean2, np.float32), np.asarray(var2, np.float32),
        np.asarray(W3, np.float32))
    q = np.asarray(query_emb, np.float32)
    k = np.asarray(key_emb, np.float32)
    sl = np.asarray(seq_length).reshape(-1).astype(np.int64)

    qW = (q @ Wq).astype(np.float32)  # [B, 64]

    order = np.argsort(sl, kind="stable")
    shards = [order[c::NCORES] for c in range(NCORES)]

    sched = []
    for g in range(NGROUPS):
        mx = max(int(sl[shards[c][16 * g : 16 * g + 16]].max()) for c in range(NCORES))
        sched.append((min(128, _ceil16(mx)), _ceil16(max(0, mx - 128))))
    sched = tuple(sched)

    if sched not in _CACHE:
        _CACHE[sched] = _build(sched)
    nc = _CACHE[sched]
    npair = BLOC // 2

    t_full = np.arange(128)[:, None]
    t_part = np.arange(128)[:, None] + 128

    in_maps = []
    for c in range(NCORES):
        rows = shards[c]
        slc = sl[rows]
        qs = q[rows]          # [256, 64]
        qWs = qW[rows]        # [256, 64]

        qp_t = np.zeros((128, npair), np.float32)
        qp_t[0:64] = qs[0::2].T
        qp_t[64:128] = qs[1::2].T

        qw1_t = np.zeros((1, 128 * npair), np.float32)
        qw1_r = qw1_t.reshape(npair, 2, 64)
        qw1_r[:, 0, :] = qWs[0::2]
        qw1_r[:, 1, :] = qWs[1::2]

        mk = np.zeros((128, 32 * NGROUPS), np.float32)
        for g in range(NGROUPS):
            sg_ = slc[16 * g : 16 * g + 16]
            full = (t_full < sg_[None, :]).astype(np.float32)   # [128, 16]
            part = (t_part < sg_[None, :]).astype(np.float32)
            mk[:, 32 * g + 0 : 32 * g + 32 : 4] = full[:, 0::2]
            mk[:, 32 * g + 1 : 32 * g + 32 : 4] = full[:, 1::2]
            mk[:, 32 * g + 2 : 32 * g + 32 : 4] = part[:, 0::2]
            mk[:, 32 * g + 3 : 32 * g + 32 : 4] = part[:, 1::2]

        in_maps.append({
            "key": np.ascontiguousarray(k[rows]).reshape(BLOC * S, D),
            "qp": qp_t,
            "qw1": qw1_t.astype(bf),
            "maskd": mk.astype(bf),
            "wk2": wk2, "wqk2": wqk2, "w2b": w2b, "w2a": w2a, "w2na": w2na,
            "w34": w34, "colsb": colsb,
        })

    res = run_bass_kernel_spmd(nc, in_maps, list(range(NCORES)), trace=TRACE)
    global LAST_RESULT
    LAST_RESULT = res

    out_full = np.zeros((B, D), np.float32)
    for c in range(NCORES):
        out_full[shards[c]] = np.asarray(res.results[c]["out"], np.float32).T
    return out_full
